# revision 26
# baseline (speedup 1.0000x reference)
"""TRN2 Bass kernel for the Acrobot GN-MPC graph-network step.

Self-contained: takes FULL unsharded inputs, shards batch B=131072 across 8
NeuronCores (pure data parallel), runs one SPMD Bass/Tile program, returns the
FULL [B, 4] output.

Per-core dataflow (Bc = 16384):
  - x/u load contiguously; a strided pad copy + DVE 32x32 stream-transpose
    put features on partitions in 4 partition groups (batch quarters).
  - The 4-layer MLP chain runs in transposed-activation layout with 512-column
    windows; all matmuls in bf16 (weights + activations; fp32 PSUM), which
    enables the fast PE weight-load path; small-K matmuls use PE row-group
    tiling, the L4 output matmuls accumulate partition-disjoint bands into one
    PSUM bank via zero-padded weights.
  - In L3, the z-only (wt) matmuls are emitted BEFORE the wn1a matmuls so the
    PE works while the L2 relu drains complete (-12us).
  - Normalizer stats + constant edge features are folded into weights/biases
    host-side; biases ride on the relu drain ops (16 on Act / 8 on DVE per
    window, empirically best split); epilogue bias-add on Act.
  - delta + bn2 drains via one Act op, inverse transpose back to natural
    layout (DVE), then the residual x + delta rides on the compact copy
    (GPSIMD tensor_tensor) so x stays exact fp32; contiguous store.
  - Numerics: all-bf16 internals give rel err 1.36e-3 vs the fp32 reference
    (gate is 2e-2); the residual path keeps x exact.
"""

import sys

if "/opt/trn_rl_repo" not in sys.path:
    sys.path.insert(0, "/opt/trn_rl_repo")

from contextlib import ExitStack

import numpy as np

import concourse.bass as bass
import concourse.bacc as bacc
import concourse.tile as tile
from concourse import mybir
from concourse._compat import with_exitstack
from concourse.bass_utils import run_bass_kernel_spmd

F32 = mybir.dt.float32
BF16 = mybir.dt.bfloat16
AF = mybir.ActivationFunctionType
ALU = mybir.AluOpType

H = 128
N_CORES = 8
B_FULL = 131072
BC = B_FULL // N_CORES  # 16384 per core

WEIGHT_SHAPES = {
    "w1e0": (128, H),
    "w1e1": (128, H),
    "wt0": (128, H),
    "wt1": (128, H),
    "we2": (H, H),
    "wn1a": (H, H),
    "wn2x0g0": (H, 128), "wn2x0g1": (H, 128), "wn2x0g2": (H, 128), "wn2x0g3": (H, 128),
    "wn2x1g0": (H, 128), "wn2x1g1": (H, 128), "wn2x1g2": (H, 128), "wn2x1g3": (H, 128),
    "be1": (128, 1),
    "be2": (128, 1),
    "bhdd": (128, 1),
    "bn2ex": (128, 1),
}


def _prep_weights(inp: dict) -> dict:
    """Fold normalizers + constant edge features into raw-input weights.

    Raw per-element feature order on partitions is [x0, x1, x2, x3, u].
    Node k features are (x_k, x_{k+2}); edge features are (u, 0, 0).
    """
    g = lambda k: np.asarray(inp[k], np.float32)
    We1, be1 = g("We1"), g("be1")
    Wn1, bn1, Wn2, bn2 = g("Wn1"), g("bn1"), g("Wn2"), g("bn2")
    nm, ns = g("node_mean"), g("node_std")
    em, es = g("edge_mean"), g("edge_std")

    w1e0 = np.zeros((128, H), np.float32)
    w1e1 = np.zeros((128, H), np.float32)
    wt0 = np.zeros((128, H), np.float32)
    wt1 = np.zeros((128, H), np.float32)
    z128 = np.zeros(H, np.float32)
    e0_rows = np.stack(
        [We1[10] / ns[0], We1[12] / ns[0], We1[11] / ns[1], We1[13] / ns[1],
         We1[14] / es[0]]
    )
    e1_rows = np.stack(
        [We1[12] / ns[0], We1[10] / ns[0], We1[13] / ns[1], We1[11] / ns[1],
         We1[14] / es[0]]
    )
    t0_rows = np.stack([Wn1[10] / ns[0], z128, Wn1[11] / ns[1], z128])
    t1_rows = np.stack([z128, Wn1[10] / ns[0], z128, Wn1[11] / ns[1]])
    for gi in range(4):
        w1e0[32 * gi : 32 * gi + 5] = e0_rows
        w1e1[32 * gi : 32 * gi + 5] = e1_rows
        wt0[32 * gi : 32 * gi + 4] = t0_rows
        wt1[32 * gi : 32 * gi + 4] = t1_rows

    be1_eff = (
        be1
        - em[1] / es[1] * We1[15]
        - em[2] / es[2] * We1[16]
        - (nm[0] / ns[0]) * (We1[10] + We1[12])
        - (nm[1] / ns[1]) * (We1[11] + We1[13])
        - (em[0] / es[0]) * We1[14]
    )
    bhdd = bn1 - (nm[0] / ns[0]) * Wn1[10] - (nm[1] / ns[1]) * Wn1[11]

    wn2pad = {}
    for gi in range(4):
        a = np.zeros((H, 128), np.float32)
        a[:, 32 * gi + 0] = Wn2[:, 0]
        a[:, 32 * gi + 2] = Wn2[:, 1]
        wn2pad[f"wn2x0g{gi}"] = a
        b = np.zeros((H, 128), np.float32)
        b[:, 32 * gi + 1] = Wn2[:, 0]
        b[:, 32 * gi + 3] = Wn2[:, 1]
        wn2pad[f"wn2x1g{gi}"] = b

    bn2ex = np.zeros((128, 1), np.float32)
    for gi in range(4):
        bn2ex[32 * gi + 0 : 32 * gi + 2, 0] = bn2[0]
        bn2ex[32 * gi + 2 : 32 * gi + 4, 0] = bn2[1]

    full = {
        "w1e0": w1e0,
        "w1e1": w1e1,
        "wt0": wt0,
        "wt1": wt1,
        "we2": np.ascontiguousarray(np.asarray(inp["We2"], np.float32)),
        "wn1a": np.ascontiguousarray(Wn1[12:140]),
        **wn2pad,
        "be1": be1_eff.reshape(128, 1),
        "be2": np.asarray(inp["be2"], np.float32).reshape(128, 1),
        "bhdd": bhdd.reshape(128, 1),
        "bn2ex": bn2ex,
    }
    wnames = [n for n in WEIGHT_SHAPES if not n.startswith("b")]
    bnames = [n for n in WEIGHT_SHAPES if n.startswith("b")]
    pad128 = lambda a, w: np.pad(a.astype(np.float32), ((0, 128 - a.shape[0]), (0, 0)))
    wblob = np.concatenate(
        [pad128(full[n], WEIGHT_SHAPES[n][1]) for n in wnames], axis=1)
    bblob = np.concatenate([full[n] for n in bnames], axis=1)
    return {"wblob": np.ascontiguousarray(wblob.astype(mybir.dt.np(BF16))),
            "bblob": np.ascontiguousarray(bblob)}


@with_exitstack
def _gn_core_kernel(
    ctx: ExitStack,
    tc: tile.TileContext,
    x_d: bass.AP,
    u_d: bass.AP,
    out_d: bass.AP,
    w_d: dict,
    mm_dt=BF16,
    relu_engines: str = "aavaavaavaavaavaavaavaav",
    iters: int = 1,
    pe_bufs: int = 7,
    d_bufs: int = 1,
    hid_bufs: int = 8,
    epi_eng: str = "a",
    l4_mode: str = "full8",
):
    nc = tc.nc
    Bc = x_d.shape[0]
    R = Bc // 128  # rows per partition in natural layout
    assert R % 16 == 0
    W = R // 16  # number of 512-column windows
    NW = 512

    mm = lambda ap: ap
    MD = mm_dt

    consts = ctx.enter_context(tc.tile_pool(name="consts", bufs=1))
    zp = ctx.enter_context(tc.tile_pool(name="zp", bufs=4))
    otp = ctx.enter_context(tc.tile_pool(name="otp", bufs=2))
    padp = ctx.enter_context(tc.tile_pool(name="padp", bufs=3))
    opadp = ctx.enter_context(tc.tile_pool(name="opadp", bufs=2))
    onatp = ctx.enter_context(tc.tile_pool(name="onatp", bufs=2))
    hidp = ctx.enter_context(tc.tile_pool(name="hidp", bufs=hid_bufs))
    natp = ctx.enter_context(tc.tile_pool(name="natp", bufs=1))
    psum = ctx.enter_context(tc.tile_pool(name="psum", bufs=pe_bufs, space="PSUM"))
    dpsum = ctx.enter_context(tc.tile_pool(name="dpsum", bufs=d_bufs, space="PSUM"))

    wnames = [n for n in WEIGHT_SHAPES if not n.startswith("b")]
    bnames = [n for n in WEIGHT_SHAPES if n.startswith("b")]
    wid = {n: WEIGHT_SHAPES[n][1] for n in wnames}
    wtot = sum(wid.values())
    wblob = consts.tile([128, wtot], MD, tag="wblob")
    bblob = consts.tile([128, len(bnames)], F32, tag="bblob")
    nc.sync.dma_start(wblob[:], w_d["wblob"][:])
    nc.sync.dma_start(bblob[:], w_d["bblob"][:])
    wsb = {}
    off = 0
    for n in wnames:
        wsb[n] = wblob[:, off : off + wid[n]]
        off += wid[n]
    for i, n in enumerate(bnames):
        wsb[n] = bblob[:, i : i + 1]

    x_nat = natp.tile([128, 4 * R], F32)
    u_nat = natp.tile([128, R], F32)
    nc.sync.dma_start(x_nat[:], x_d.rearrange("(p r) f -> p (r f)", p=128))
    nc.sync.dma_start(u_nat[:], u_d.rearrange("(p r) f -> p (r f)", p=128))
    x_nat3 = x_nat.rearrange("p (r f) -> p r f", f=4)

    def relu(dst, src_psum, bias, i, width=NW):
        eng = relu_engines[i % len(relu_engines)]
        if eng == "v":
            nc.vector.tensor_scalar(
                dst, src_psum, bias[:, 0:1], 0.0, op0=ALU.add, op1=ALU.max
            )
        else:
            nc.scalar.activation(
                dst, src_psum, AF.Relu, bias=bias[:, 0:1], scale=1.0
            )

    out_r = out_d.rearrange("(p r) f -> p (r f)", p=128)

    def z_prep(tau):
        pad = padp.tile([128, NW], MD, name=f"pad_{tau}")
        pad3 = pad.rearrange("p (c q) -> p c q", q=32)
        nc.gpsimd.tensor_copy(pad3[:, :, 0:4], x_nat3[:, 16 * tau : 16 * tau + 16, :])
        nc.gpsimd.tensor_copy(
            pad3[:, :, 4:5],
            u_nat[:, 16 * tau : 16 * tau + 16].rearrange("p (c o) -> p c o", o=1),
        )
        zt = zp.tile([128, NW], MD, tag="z", name=f"z_{tau}")
        nc.vector.transpose(zt[:], pad[:])
        return zt

    def one_pass():
        ri = 0
        znext = z_prep(0)
        for tau in range(W):
            z = znext
            if tau + 1 < W:
                znext = z_prep(tau + 1)

            delta = dpsum.tile([128, NW], F32)

            pe0, pe1, e0h, e1h = {}, {}, {}, {}
            pa, pb, agg0, agg1 = {}, {}, {}, {}
            ph0, ph1, h0, h1 = {}, {}, {}, {}
            for g in range(4):
                b = 32 * g
                pe0[g] = psum.tile([128, NW], F32, tag="pe", name=f"pe0_{g}")
                pe1[g] = psum.tile([128, NW], F32, tag="pe", name=f"pe1_{g}")
                nc.tensor.matmul(
                    pe0[g][:], mm(wsb["w1e0"][b : b + 5, :]), mm(z[b : b + 5, :]),
                    start=True, stop=True, tile_position=(b, 0),
                )
                nc.tensor.matmul(
                    pe1[g][:], mm(wsb["w1e1"][b : b + 5, :]), mm(z[b : b + 5, :]),
                    start=True, stop=True, tile_position=(b, 0),
                )
            for g in range(4):
                e0h[g] = hidp.tile([128, NW], MD, tag="hid", name=f"e0h_{g}")
                e1h[g] = hidp.tile([128, NW], MD, tag="hid", name=f"e1h_{g}")
                relu(e0h[g][:], pe0[g][:], wsb["be1"], ri); ri += 1
                relu(e1h[g][:], pe1[g][:], wsb["be1"], ri); ri += 1
            for g in range(4):
                pa[g] = psum.tile([128, NW], F32, tag="pe", name=f"pa_{g}")
                pb[g] = psum.tile([128, NW], F32, tag="pe", name=f"pb_{g}")
                nc.tensor.matmul(pa[g][:], mm(wsb["we2"][:]), mm(e0h[g][:]), start=True, stop=True)
                nc.tensor.matmul(pb[g][:], mm(wsb["we2"][:]), mm(e1h[g][:]), start=True, stop=True)
            for g in range(4):
                agg0[g] = hidp.tile([128, NW], MD, tag="hid", name=f"agg0_{g}")
                agg1[g] = hidp.tile([128, NW], MD, tag="hid", name=f"agg1_{g}")
                relu(agg0[g][:], pb[g][:], wsb["be2"], ri); ri += 1  # agg(node0)=relu(L2(edge1))
                relu(agg1[g][:], pa[g][:], wsb["be2"], ri); ri += 1
            for g in range(4):
                b = 32 * g
                ph0[g] = psum.tile([128, NW], F32, tag="pe", name=f"ph0_{g}")
                ph1[g] = psum.tile([128, NW], F32, tag="pe", name=f"ph1_{g}")
                nc.tensor.matmul(
                    ph0[g][:], mm(wsb["wt0"][b : b + 4, :]), mm(z[b : b + 4, :]),
                    start=True, stop=False, tile_position=(b, 0),
                )
                nc.tensor.matmul(
                    ph1[g][:], mm(wsb["wt1"][b : b + 4, :]), mm(z[b : b + 4, :]),
                    start=True, stop=False, tile_position=(b, 0),
                )
                nc.tensor.matmul(ph0[g][:], mm(wsb["wn1a"][:]), mm(agg0[g][:]), start=False, stop=True)
                nc.tensor.matmul(ph1[g][:], mm(wsb["wn1a"][:]), mm(agg1[g][:]), start=False, stop=True)
            for g in range(4):
                h0[g] = hidp.tile([128, NW], MD, tag="hid", name=f"h0_{g}")
                h1[g] = hidp.tile([128, NW], MD, tag="hid", name=f"h1_{g}")
                relu(h0[g][:], ph0[g][:], wsb["bhdd"], ri); ri += 1
                relu(h1[g][:], ph1[g][:], wsb["bhdd"], ri); ri += 1
            for g in range(4):
                if l4_mode == "band":
                    b = 32 * g
                    nc.tensor.matmul(
                        delta[b : b + 32, :], mm(wsb[f"wn2x0g{g}"][:, b : b + 32]), mm(h0[g][:]),
                        start=True, stop=False, tile_position=(0, b),
                    )
                    nc.tensor.matmul(
                        delta[b : b + 32, :], mm(wsb[f"wn2x1g{g}"][:, b : b + 32]), mm(h1[g][:]),
                        start=False, stop=True, tile_position=(0, b),
                    )
                else:
                    nc.tensor.matmul(
                        delta[:, :], mm(wsb[f"wn2x0g{g}"][:]), mm(h0[g][:]),
                        start=(g == 0), stop=False,
                    )
                    nc.tensor.matmul(
                        delta[:, :], mm(wsb[f"wn2x1g{g}"][:]), mm(h1[g][:]),
                        start=False, stop=(g == 3),
                    )

            o_t = otp.tile([128, NW], MD)
            if epi_eng == "a":
                nc.scalar.activation(
                    o_t[:], delta[:], AF.Identity, bias=wsb["bn2ex"][:, 0:1], scale=1.0
                )
            else:
                nc.vector.tensor_scalar(
                    o_t[:], delta[:], wsb["bn2ex"][:, 0:1], None, op0=ALU.add
                )
            opad = opadp.tile([128, NW], MD)
            nc.vector.transpose(opad[:], o_t[:])
            onat = onatp.tile([128, 64], F32)
            opad3 = opad.rearrange("p (c q) -> p c q", q=32)
            nc.gpsimd.tensor_tensor(
                onat.rearrange("p (c f) -> p c f", f=4), opad3[:, :, 0:4],
                x_nat3[:, 16 * tau : 16 * tau + 16, :], op=ALU.add,
            )
            nc.sync.dma_start(out_r[:, 64 * tau : 64 * tau + 64], onat[:])

    if iters == 1:
        one_pass()
    else:
        with tc.For_i(0, iters, 1):
            one_pass()



_CACHED = {}


def _build_nc(mm_dt=BF16, relu_engines: str = "aavaavaavaavaavaavaavaav",
              iters: int = 1, **kw):
    key = (str(mm_dt), relu_engines, iters, tuple(sorted(kw.items())))
    if key in _CACHED:
        return _CACHED[key]
    nc = bacc.Bacc("TRN2", target_bir_lowering=False, debug=False)
    x_d = nc.declare_dram_parameter("x", [BC, 4], F32, isOutput=False)
    u_d = nc.declare_dram_parameter("u", [BC, 1], F32, isOutput=False)
    wnames = [n for n in WEIGHT_SHAPES if not n.startswith("b")]
    bnames = [n for n in WEIGHT_SHAPES if n.startswith("b")]
    wtot = sum(WEIGHT_SHAPES[n][1] for n in wnames)
    w_aps = {
        "wblob": nc.declare_dram_parameter("wblob", [128, wtot], mm_dt, isOutput=False),
        "bblob": nc.declare_dram_parameter("bblob", [128, len(bnames)], F32, isOutput=False),
    }
    out_d = nc.declare_dram_parameter("out", [BC, 4], F32, isOutput=True)
    with tile.TileContext(nc) as tc:
        _gn_core_kernel(tc, x_d, u_d, out_d, w_aps, mm_dt=mm_dt,
                        relu_engines=relu_engines, iters=iters, **kw)
    nc.compile()
    _CACHED[key] = nc
    return nc


def run_sharded(x, u, wd, mm_dt=BF16, relu_engines="aavaavaavaavaavaavaavaav",
                trace=False):
    """Shard, dispatch to 8 cores, gather. Returns (out, BassKernelResults)."""
    x = np.ascontiguousarray(np.asarray(x, np.float32))
    u = np.ascontiguousarray(np.asarray(u, np.float32))
    nc = _build_nc(mm_dt, relu_engines)
    in_maps = []
    for c in range(N_CORES):
        m = {"x": x[c * BC : (c + 1) * BC], "u": u[c * BC : (c + 1) * BC]}
        m.update(wd)
        in_maps.append(m)
    res = run_bass_kernel_spmd(nc, in_maps, list(range(N_CORES)), trace=trace)
    out = np.concatenate([r["out"] for r in res.results], axis=0)
    return out, res


def kernel(**inputs) -> np.ndarray:
    wd = _prep_weights(inputs)
    out, _ = run_sharded(inputs["x"], inputs["u"], wd)
    return out.astype(np.float32)


def make_runner(nc):
    """Build the 8-core sharded jit callable once (mimics run_bass_via_pjrt)
    so repeated timed invocations skip re-tracing."""
    import jax
    from jax.sharding import Mesh, PartitionSpec
    from jax.experimental.shard_map import shard_map
    from concourse import bass2jax, mybir as mb
    from concourse.bass2jax import _bass_exec_p, install_neuronx_cc_hook

    install_neuronx_cc_hook()
    n_cores = N_CORES
    in_names, out_names, out_avals, zero_outs = [], [], [], []
    partition_name = nc.partition_id_tensor.name if nc.partition_id_tensor else None
    for alloc in nc.m.functions[0].allocations:
        if not isinstance(alloc, mb.MemoryLocationSet):
            continue
        name = alloc.memorylocations[0].name
        if alloc.kind == "ExternalInput":
            if name != partition_name:
                in_names.append(name)
        elif alloc.kind == "ExternalOutput":
            shape = tuple(alloc.tensor_shape)
            dtype = mb.dt.np(alloc.dtype)
            out_names.append(name)
            out_avals.append(jax.core.ShapedArray(shape, dtype))
            zero_outs.append(np.zeros(shape, dtype))
    n_params = len(in_names)
    n_outs = len(out_avals)
    in_names_all = in_names + out_names
    if partition_name is not None:
        in_names_all = in_names_all + [partition_name]
    donate = tuple(range(n_params, n_params + n_outs))

    def _body(*args):
        operands = list(args)
        if partition_name is not None:
            operands.append(bass2jax.partition_id_tensor())
        outs = _bass_exec_p.bind(
            *operands,
            out_avals=tuple(out_avals),
            in_names=tuple(in_names_all),
            out_names=tuple(out_names),
            lowering_input_output_aliases=(),
            sim_require_finite=True,
            sim_require_nnan=True,
            nc=nc,
        )
        return tuple(outs)

    devices = jax.devices()[:n_cores]
    mesh = Mesh(np.asarray(devices), ("core",))
    in_specs = (PartitionSpec("core"),) * (n_params + n_outs)
    out_specs = (PartitionSpec("core"),) * n_outs
    sharded = jax.jit(
        shard_map(_body, mesh=mesh, in_specs=in_specs, out_specs=out_specs,
                  check_rep=False),
        donate_argnums=donate, keep_unused=True,
    )

    def run(in_maps, timeit=0):
        import time as _t
        per_core = [[np.asarray(m[n]) for n in in_names] for m in in_maps]
        concat_in = [
            np.concatenate([per_core[c][i] for c in range(n_cores)], axis=0)
            for i in range(n_params)
        ]
        concat_zeros = [
            np.zeros((n_cores * z.shape[0], *z.shape[1:]), z.dtype)
            for z in zero_outs
        ]
        out_arrs = jax.block_until_ready(sharded(*concat_in, *concat_zeros))
        times = []
        for _ in range(timeit):
            cz = [np.zeros_like(z) for z in concat_zeros]
            t0 = _t.perf_counter()
            out_arrs2 = jax.block_until_ready(sharded(*concat_in, *cz))
            times.append(_t.perf_counter() - t0)
            del out_arrs2
        results = [
            {n: np.asarray(out_arrs[i]).reshape(n_cores, *out_avals[i].shape)[c]
             for i, n in enumerate(out_names)}
            for c in range(n_cores)
        ]
        return results, times
    return run


def _build_null_nc():
    """Same I/O signature, trivial work — for dispatch-overhead subtraction."""
    if "null" in _CACHED:
        return _CACHED["null"]
    nc = bacc.Bacc("TRN2", target_bir_lowering=False, debug=False)
    x_d = nc.declare_dram_parameter("x", [BC, 4], F32, isOutput=False)
    u_d = nc.declare_dram_parameter("u", [BC, 1], F32, isOutput=False)
    w_aps = {
        name: nc.declare_dram_parameter(
            name, list(shp), F32 if name.startswith("b") else mybir.dt.float32r,
            isOutput=False)
        for name, shp in WEIGHT_SHAPES.items()
    }
    out_d = nc.declare_dram_parameter("out", [BC, 4], F32, isOutput=True)
    with tile.TileContext(nc) as tc:
        with ExitStack() as ctx:
            p = ctx.enter_context(tc.tile_pool(name="p", bufs=1))
            t = p.tile([128, 512], F32)
            nc.sync.dma_start(t[:], x_d.rearrange("(p r) f -> p (r f)", p=128))
            nc.sync.dma_start(out_d.rearrange("(p r) f -> p (r f)", p=128), t[:])
    nc.compile()
    _CACHED["null"] = nc
    return nc

